# revision 46
# baseline (speedup 1.0000x reference)
"""IterSpatialCorrelationSampler (P=9, DP=1) Trainium2 Bass kernel.

out[b,i,j,y,x] = sum_c in1[b,c,y,x] * pad(in2)[b,c,y+i,x+j]   (pad=4 each side)

Strategy:
  - 8 cores, each handles (b, yhalf): b = core//2, 48 rows of y.
  - TensorE Gram-band formulation with a 16y x 8x m-tile (128 output
    positions on PSUM partitions, p = ly*8+lx) against a 24y x 16x window
    of padded in2 (n = 384 free), contraction over c = 2 accumulating
    k=128 fp8 matmuls per tile (96 total, ~165ns each warm).  For
    partition p the 81 useful values sit at band[p, 16*ly+lx + 16*di+dj];
    the host slices them out (cheap numpy gather).
  - Inputs quantized host-side to fp8 e3m4 (rel err 1.8e-2, under the
    2e-2 bar); PSUM accumulates fp32; band ships to HBM as f16.
  - PSUM tiles are bank PAIRS (4 in flight); one DVE/ACT copy per pair,
    alternating engines; the last four pairs split into two parallel
    single-tile copies each (they gate the final stores).
  - Stores: 13 full-band 4-tile chunks, each ONE dma_start on the sync
    queue with 128 descriptors of 3KB contiguous-per-partition runs
    (full SDMA rate; sub-2KB descriptors run at ~half rate, and every
    extra compact-store dma_start costs 0.6-1.5us of issuing-engine
    time, which is why compact-span stores lose despite moving 2.4x
    fewer bytes).  A chunk fires the moment its pair of copies lands
    (every ~1.4us), keeping the SDMA store stream continuous instead
    of piling up behind coarse chunk gates; only the last 2 tiles
    (~0.2MB) drain after the final copy.
  - Loads in deadline order alternating the two HWDGE queues, with the
    first-matmul gates (in2 ch0 rows 0:24, in1 tiles 0:8) first on the
    sync queue (the scalar queue's DGE is blocked ~1.3us at start by
    the framework ACT_TABLE_LOAD).  SDMA data cannot move before ~8us
    (fixed framework preamble); 7 dummy matmuls plus one filler matmul
    per ch0-pass tile keep the PE busy so the HAM clock gate reaches
    2.4 GHz right as real data lands and never re-throttles.
  - ty0 runs a ch0-only pass over its first 8 tiles so the PE has work
    before the ch1 image chunk arrives.
"""

import numpy as np

import concourse.bass as bass
import concourse.bacc as bacc
import concourse.tile as tile
import concourse.mybir as mybir
from concourse.bass_utils import run_bass_kernel_spmd

# problem constants (hardcoded per contract)
B, C, H, W = 4, 256, 96, 128
P = 9
OFF = 4
NCORES = 8
YH = H // 2          # 48 rows per core
WP = W + 2 * OFF     # 136
ROWS = YH + 2 * OFF  # 56 rows of padded in2 per core
MT_Y, MT_X = 16, 8   # m-tile shape (16y x 8x = 128 partitions)
NW_Y, NW_X = MT_Y + P - 1, MT_X + P - 1   # 24 x 16 window
NTY, NTX = YH // MT_Y, W // MT_X          # 3 x 16 = 48 tiles
NT = NTY * NTX
NFREE = NW_Y * NW_X                       # 384
_F8 = mybir.dt.np(mybir.dt.float8e3)   # ml_dtypes.float8_e3m4

_cached = {}


def _build():
    nc = bacc.Bacc(
        "TRN2",
        target_bir_lowering=False,
        debug=False,
        enable_asserts=False,
        num_devices=NCORES,
    )
    f16 = mybir.dt.float16
    f32 = mybir.dt.float32
    f8 = mybir.dt.float8e3

    in1_d = nc.dram_tensor("in1t", [128, NT, 2, MT_Y * MT_X], f8, kind="ExternalInput").ap()
    in2_d = nc.dram_tensor("in2c", [128, 2, ROWS, WP], f8, kind="ExternalInput").ap()
    band_d = nc.dram_tensor("band", [128, NT, NFREE], f16, kind="ExternalOutput").ap()

    with tile.TileContext(nc) as tc:
        with (
            tc.tile_pool(name="sb", bufs=1) as sb,
            tc.tile_pool(name="ps", bufs=4, space="PSUM") as ps,
        ):
            in2_sb = sb.tile([128, 2, ROWS, WP], f8, name="in2sb")
            in1_sb = sb.tile([128, NT, 2, MT_Y * MT_X], f8, name="in1sb")
            band = sb.tile([128, NT, NFREE], f16, name="band")

            # Loads in deadline order, alternating the two HWDGE queues.
            # (in1 16:32 and in2 rows 24:40 are both due when ty1 starts
            # at ~17us; the ty2 chunks are due ~23us.)
            nc.sync.dma_start(out=in2_sb[:, 0, 0:24, :], in_=in2_d[:, 0, 0:24, :])
            nc.sync.dma_start(out=in1_sb[:, 0:8, :, :], in_=in1_d[:, 0:8, :, :])
            nc.scalar.dma_start(out=in1_sb[:, 8:16, :, :], in_=in1_d[:, 8:16, :, :])
            nc.sync.dma_start(out=in2_sb[:, 1, 0:24, :], in_=in2_d[:, 1, 0:24, :])
            nc.sync.dma_start(out=in2_sb[:, 0, 24:40, :], in_=in2_d[:, 0, 24:40, :])
            nc.scalar.dma_start(out=in2_sb[:, 1, 24:40, :], in_=in2_d[:, 1, 24:40, :])
            nc.sync.dma_start(out=in1_sb[:, 16:32, :, :], in_=in1_d[:, 16:32, :, :])
            nc.scalar.dma_start(out=in2_sb[:, 0, 40:ROWS, :], in_=in2_d[:, 0, 40:ROWS, :])
            nc.sync.dma_start(out=in1_sb[:, 32:NT, :, :], in_=in1_d[:, 32:NT, :, :])
            nc.scalar.dma_start(out=in2_sb[:, 1, 40:ROWS, :], in_=in2_d[:, 1, 40:ROWS, :])

            # Warm up the PE until the first real data lands (~12.5us):
            # HAM clock ramp 1.2 -> 2.4 GHz needs ~3.4us sustained activity,
            # and any >3.4us idle re-throttles.
            wu = sb.tile([128, 512], f8, name="wu")
            nc.vector.memset(wu[:, :], 0.0)
            wpt = ps.tile([128, 2, 512], f32, tag="pt", name="wpt")
            for i in range(7):
                nc.tensor.matmul(
                    wpt[:, i % 2, :], wu[:, 0:128], wu[:, :], start=True, stop=True
                )

            def win_ap(ch, ty, tx):
                return in2_sb[
                    :, ch,
                    MT_Y * ty : MT_Y * ty + NW_Y,
                    MT_X * tx : MT_X * tx + NW_X,
                ]

            def copy_pair(tp, pt):
                # one PSUM->SBUF f16 copy per bank pair, alternating engines;
                # the last two pairs (critical tail) split into parallel
                # single-tile copies, and pairs 20/21 stay whole so both
                # engines are free the moment the final pairs' matmuls end.
                if tp >= 22:
                    t0 = 2 * tp
                    nc.vector.tensor_copy(band[:, t0 : t0 + 1, :], pt[:, 0:1, 0:NFREE])
                    nc.scalar.mul(band[:, t0 + 1 : t0 + 2, :], pt[:, 1:2, 0:NFREE], 1.0)
                elif tp % 2 == 0:
                    nc.vector.tensor_copy(
                        band[:, 2 * tp : 2 * tp + 2, :], pt[:, :, 0:NFREE]
                    )
                else:
                    nc.scalar.mul(
                        band[:, 2 * tp : 2 * tp + 2, :], pt[:, :, 0:NFREE], 1.0
                    )

            def store_full(t0, t1, eng):
                # one DMA, 128 descriptors of (t1-t0)*768B contiguous per
                # partition: full SDMA rate (the RMW knee is ~2KB), and
                # near-zero issue cost.  Ships junk band columns, but only
                # while the SDMA engines are otherwise underused.
                eng.dma_start(out=band_d[:, t0:t1, :], in_=band[:, t0:t1, :])

            def mm(pt, j, t, ch, ty, tx, start, stop):
                nc.tensor.matmul(
                    pt[:, j, 0:NFREE], in1_sb[:, t, ch, :], win_ap(ch, ty, tx),
                    start=start, stop=stop,
                )

            for ty in range(NTY):
                if ty == 0:
                    # ch0-only pass over tx 0..7 (ch1 image lands later).
                    # The pass is paced by the arriving loads, so keep the
                    # PE busy (HAM warm) with a filler matmul per tile.
                    pts = []
                    for tp in range(4):
                        pt = ps.tile([128, 2, 512], f32, tag="pt", name=f"pt0_{tp}")
                        pts.append(pt)
                        for j in range(2):
                            if tp < 2:
                                nc.tensor.matmul(
                                    wpt[:, j, 0:384], wu[:, 0:128], wu[:, 0:384],
                                    start=True, stop=True,
                                )
                            mm(pt, j, 2 * tp + j, 0, 0, 2 * tp + j, True, False)
                    for tp in range(4):
                        for j in range(2):
                            mm(pts[tp], j, 2 * tp + j, 1, 0, 2 * tp + j, False, True)
                        copy_pair(tp, pts[tp])
                        if tp % 2 == 1:
                            store_full(2 * tp - 2, 2 * tp + 2, nc.sync)
                    rng = range(4, 8)
                else:
                    rng = range(8)
                for tp in rng:
                    tpg = ty * 8 + tp
                    pt = ps.tile([128, 2, 512], f32, tag="pt", name=f"pt{tpg}")
                    for j in range(2):
                        t = 2 * tpg + j
                        tx = t - ty * NTX
                        for ch in range(2):
                            mm(pt, j, t, ch, ty, tx, ch == 0, ch == 1)
                    copy_pair(tpg, pt)
                    if tpg % 2 == 1 and tpg <= 21:
                        # 4-tile chunk as soon as its pair of copies lands:
                        # 3072B descriptors (full rate), fired every ~1.4us
                        # so the store stream never starves the SDMA engines
                        store_full(2 * tpg - 2, 2 * tpg + 2, nc.sync)
                    elif tpg == 22:
                        store_full(44, 46, nc.sync)
            store_full(46, NT, nc.sync)

    nc.compile()
    return nc


def _prep_inputs(input1, input2):
    """Build per-core input maps (fp8, padded, tiled, c split on partitions)."""
    in_maps = []
    pad2 = np.pad(
        np.asarray(input2), ((0, 0), (0, 0), (OFF, OFF), (OFF, OFF))
    )  # [B, C, H+8, WP]
    a1 = np.asarray(input1)
    for core in range(NCORES):
        b, yh = core // 2, core % 2
        y0 = yh * YH
        i1 = a1[b, :, y0 : y0 + YH, :].reshape(2, 128, NTY, MT_Y, NTX, MT_X)
        # -> [p, ty, tx, ch, ly, lx] -> [128, NT, 2, 128]
        i1 = i1.transpose(1, 2, 4, 0, 3, 5).reshape(128, NT, 2, MT_Y * MT_X)
        p2 = pad2[b, :, y0 : y0 + ROWS, :].reshape(2, 128, ROWS, WP)
        i2c = p2.transpose(1, 0, 2, 3).astype(_F8)
        in_maps.append(
            {
                "in1t": np.ascontiguousarray(i1.astype(_F8)),
                "in2c": np.ascontiguousarray(i2c),
            }
        )
    return in_maps


_OFFS = np.array([16 * di + dj for di in range(P) for dj in range(P)])


def _extract(band):
    """band [128, NT, 384] f16 -> out_local [9, 9, 48, 128] f32.

    Partition p = ly*8+lx: out[di,dj] = band[p, t, 16*ly + lx + 16*di + dj].
    """
    b = band.reshape(MT_Y, MT_X, NTY, NTX, NFREE).astype(np.float32)
    sel = np.empty((MT_Y, MT_X, NTY, NTX, P * P), dtype=np.float32)
    for ly in range(MT_Y):
        for lx in range(MT_X):
            sel[ly, lx] = b[ly, lx][:, :, 16 * ly + lx + _OFFS]
    out = sel.reshape(MT_Y, MT_X, NTY, NTX, P, P).transpose(4, 5, 2, 0, 3, 1)
    return np.ascontiguousarray(out).reshape(P, P, YH, W)


def run(input1, input2, trace=False, **trace_kwargs):
    if "nc" not in _cached:
        _cached["nc"] = _build()
    nc = _cached["nc"]
    in_maps = _prep_inputs(input1, input2)
    res = run_bass_kernel_spmd(
        nc, in_maps, list(range(NCORES)), trace=trace, **trace_kwargs
    )
    out = np.empty((B, P, P, H, W), dtype=np.float32)
    for core in range(NCORES):
        b, yh = core // 2, core % 2
        band = res.results[core]["band"]
        out[b, :, :, yh * YH : (yh + 1) * YH, :] = _extract(band)
    return out, res


def kernel(input1, input2):
    out, _ = run(input1, input2, trace=False)
    return out
